# revision 1
# baseline (speedup 1.0000x reference)
"""Trainium2 Bass kernel for nn_Attention_79164837199973.

Bias-augmented multi-head self-attention with sigmoid gating.
B=4, N=1024, CQ=CH=512, H=8, D=64.

Sharding (8 cores, no collectives): core c -> batch b=c//2, query-row half
r=c%2 (512 rows). Each core computes k/v projections for the full sequence
of its batch (duplicated across the 2 cores of a batch pair -- cheaper than
an all-reduce), attention for all 8 heads over its 512 query rows, then
to_out + gating. Per-core outputs are exact disjoint shards of the result.

Layout notes (all matmuls bf16, f32 PSUM accumulate):
  - every DRAM input is host-pre-swizzled to its exact SBUF layout
    ([128 partitions, ...] with contiguous per-partition bytes) so each
    dma_start lowers to ~16 fat descriptors instead of ~1000 strided ones.
  - scores computed transposed: sT[nk,nq] via matmul(lhsT=kT[d,nk_chunk],
    rhs=qT[d,nq]); K=d=64, two heads row-packed into the PE array
    (partitions 0-63 / 64-127 concurrently).
  - softmax without max-subtraction (logits bounded ~+-7 here):
    p = exp(qk) * exp(bias), exp(bias) precomputed on host (bf16),
    multiplied in on VectorE (bf16 packed mode).
  - denominator via a ones-column appended to V (M=65 PV matmul, row 64);
    reciprocals batched 4 heads at a time into [4,512] DVE reciprocals
    (reciprocal costs ~6.5 cyc per per-lane element regardless of lane
    count, so 8 separate [1,512] ops would cost 8x).
  - per-head 1/denom broadcast over 64 partitions via tiny K=8 PE matmuls
    against a [8, 8*64] one-hot selector.

Scheduling rules learned on silicon (Tile queues are strict FIFO per
engine, priority = emission order):
  - ALL projection matmuls + their PSUM evacuations are emitted before
    attention pair 0, so no evacuation's PSUM-completion wait ever
    head-of-line blocks the attention-phase VectorE mult stream.
  - PSUM evacuations sit on the engine whose queue carries the upstream
    dependency (oraw on DVE after the mults, not ACT).
  - softmax-denominator normalization is split: the gather DMA is issued
    at the pair boundary, but the reciprocal+broadcast chain is emitted
    after the NEXT pair's first mults so the DMA latency is hidden.
  - at the tail, every ready-dependency chain (gate tanh, to_out ko0-1)
    is emitted before the final norm batch to fill its dead time.
  - pair 0 is split-phase: its logits+exps are emitted right after
    kt0/qt0 (ScalarE busy from ~13us instead of ~28us), with the
    DVE mults + PV matmuls deferred until after all projection
    evacuations; the held e-tiles live in a DEDICATED pool so later
    pairs' exp pipeline never starves on slot recycling.
"""

import os
import sys

sys.path.insert(0, "/opt/trn_rl_repo")

import numpy as np

import concourse.bass as bass
import concourse.tile as tile
from concourse import bacc, mybir

B, N, CQ, CH, H = 4, 1024, 512, 512, 8
D = CH // H  # 64
NQ = N // 2  # 512 query rows per core
P = 128
F32 = mybir.dt.float32
BF16 = mybir.dt.bfloat16
AF = mybir.ActivationFunctionType


def build_nc():
    nc = bacc.Bacc("TRN2", target_bir_lowering=False, debug=False, num_devices=8)

    # ---- DRAM parameters, already in SBUF layout (host pre-swizzled) ----
    xt_e = nc.declare_dram_parameter("xt", [P, 4, N], BF16, isOutput=False)
    xtq_e = nc.declare_dram_parameter("xtq", [P, 4, NQ], BF16, isOutput=False)
    ebt_e = nc.declare_dram_parameter("ebt", [P, H * 8, NQ], BF16, isOutput=False)
    wqt_e = nc.declare_dram_parameter("wqt", [P, 4, CH], BF16, isOutput=False)
    wkt_e = nc.declare_dram_parameter("wkt", [P, 4, CH], BF16, isOutput=False)
    wvt_e = nc.declare_dram_parameter("wvt", [P, 4, CH], BF16, isOutput=False)
    wot_e = nc.declare_dram_parameter("wot", [P, 4, CQ], BF16, isOutput=False)
    wgt_e = nc.declare_dram_parameter("wgt", [P, 4, CQ], BF16, isOutput=False)
    bqs_e = nc.declare_dram_parameter("bqs", [P, 4], F32, isOutput=False)
    bo_e = nc.declare_dram_parameter("bo", [P, 4], F32, isOutput=False)
    gb_e = nc.declare_dram_parameter("gb", [P, 4], F32, isOutput=False)
    sel8_e = nc.declare_dram_parameter("sel8", [H, H * D], BF16, isOutput=False)
    out_e = nc.declare_dram_parameter("out", [P, 4, NQ], F32, isOutput=True)

    with tile.TileContext(nc) as tc:
        with (
            tc.tile_pool(name="singles", bufs=1) as singles,
            tc.tile_pool(name="etmp", bufs=3) as etmp,
            tc.tile_pool(name="vtmp", bufs=2) as vtmp,
            tc.tile_pool(name="ps_s", bufs=2, space="PSUM") as ps_s,
            tc.tile_pool(name="ps_sm", bufs=4, space="PSUM") as ps_sm,
        ):
            # ---- persistent SBUF tiles ----
            xt_sb = singles.tile([P, 4, N], BF16)
            xtq_sb = singles.tile([P, 4, NQ], BF16)
            wqt_sb = singles.tile([P, 4, CH], BF16)
            wkt_sb = singles.tile([P, 4, CH], BF16)
            wvt_sb = singles.tile([P, 4, CH], BF16)
            wot_sb = singles.tile([P, 4, CQ], BF16)
            wgt_sb = singles.tile([P, 4, CQ], BF16)
            bqs_sb = singles.tile([P, 4], F32)
            bo_sb = singles.tile([P, 4], F32)
            gb_sb = singles.tile([P, 4], F32)
            sel8_sb = singles.tile([H, H * D], BF16)
            ebt_sb = singles.tile([P, H * 8, NQ], BF16)
            kt_sb = singles.tile([P, 4, N], BF16)
            qt_sb = singles.tile([P, 4, NQ], BF16)
            vaug_sb = singles.tile([P, 8, H * (D + 1)], BF16)
            oraw_sb = singles.tile([D + 1, H, NQ], BF16)  # 0-63 o, row 64 den
            osc_sb = singles.tile([D, H, NQ], BF16)  # normalized o, all heads
            ofin_sb = singles.tile([P, 4, NQ], BF16)  # head-merged o^T
            z_sb = singles.tile([P, 4, NQ], BF16)  # gate pre-activation
            gate_sb = singles.tile([P, 4, NQ], BF16)
            outf_sb = singles.tile([P, 4, NQ], F32)
            warm_sb = singles.tile([1, 8], F32)
            warmo_sb = singles.tile([1, 8], BF16)

            # force the exp table load off the critical path (first ACT op)
            nc.vector.memset(warm_sb, 0.0)
            nc.scalar.activation(out=warmo_sb, in_=warm_sb, func=AF.Exp)

            # ---- input DMAs: one HWDGE ring; FIFO order = priority order
            def ebt_load(h):
                nc.sync.dma_start(
                    out=ebt_sb[:, h * 8 : (h + 1) * 8, :],
                    in_=ebt_e[:, h * 8 : (h + 1) * 8, :],
                )

            nc.sync.dma_start(out=xt_sb, in_=xt_e[:, :, :])
            nc.sync.dma_start(out=wkt_sb, in_=wkt_e[:, :, :])
            nc.sync.dma_start(out=wqt_sb, in_=wqt_e[:, :, :])
            nc.sync.dma_start(out=xtq_sb, in_=xtq_e[:, :, :])
            nc.sync.dma_start(out=bqs_sb, in_=bqs_e[:, :])
            nc.sync.dma_start(out=wvt_sb, in_=wvt_e[:, :, :])
            for h in range(4):
                ebt_load(h)
            nc.sync.dma_start(out=wgt_sb, in_=wgt_e[:, :, :])
            nc.sync.dma_start(out=gb_sb, in_=gb_e[:, :])
            nc.sync.dma_start(out=sel8_sb, in_=sel8_e[:, :])
            for h in range(4, H):
                ebt_load(h)
            nc.sync.dma_start(out=wot_sb, in_=wot_e[:, :, :])
            nc.sync.dma_start(out=bo_sb, in_=bo_e[:, :])

            # ones column of v_aug (col D of each head's 65-wide group)
            nc.vector.memset(
                vaug_sb.rearrange("p c (h e) -> p c h e", h=H)[:, :, :, D : D + 1],
                1.0,
            )

            def kt_proj(mo, act_evac=False):
                for no in range(2):
                    ps = ps_sm.tile([P, 512], F32, tag="ps", name="ps_k")
                    for ko in range(4):
                        nc.tensor.matmul(
                            ps,
                            lhsT=wkt_sb[:, ko, mo * P : (mo + 1) * P],
                            rhs=xt_sb[:, ko, no * 512 : (no + 1) * 512],
                            start=(ko == 0),
                            stop=(ko == 3),
                        )
                    dst = kt_sb[:, mo, no * 512 : (no + 1) * 512]
                    if act_evac:
                        nc.scalar.copy(out=dst, in_=ps)
                    else:
                        nc.vector.tensor_copy(out=dst, in_=ps)

            def qt_proj(mo, act_evac=False):
                ps = ps_sm.tile([P, 512], F32, tag="ps", name="ps_q")
                for ko in range(4):
                    nc.tensor.matmul(
                        ps,
                        lhsT=wqt_sb[:, ko, mo * P : (mo + 1) * P],
                        rhs=xtq_sb[:, ko, :],
                        start=(ko == 0),
                        stop=(ko == 3),
                    )
                if act_evac:
                    nc.scalar.activation(
                        out=qt_sb[:, mo, :],
                        in_=ps,
                        func=AF.Identity,
                        bias=bqs_sb[:, mo : mo + 1],
                    )
                else:
                    nc.vector.tensor_scalar_add(
                        out=qt_sb[:, mo, :], in0=ps, scalar1=bqs_sb[:, mo : mo + 1]
                    )

            def v_proj(c, act_evac=False):
                ps = ps_sm.tile([P, 512], F32, tag="ps", name="ps_v")
                for ko in range(4):
                    nc.tensor.matmul(
                        ps,
                        lhsT=xt_sb[:, ko, c * P : (c + 1) * P],
                        rhs=wvt_sb[:, ko, :],
                        start=(ko == 0),
                        stop=(ko == 3),
                    )
                dst = vaug_sb.rearrange("p c (h e) -> p c h e", h=H)[:, c, :, 0:D]
                srcv = ps.rearrange("p (h d) -> p h d", h=H)
                if act_evac:
                    nc.scalar.copy(out=dst, in_=srcv)
                else:
                    nc.vector.tensor_copy(out=dst, in_=srcv)

            def gate_proj(mo):
                ps = ps_sm.tile([P, 512], F32, tag="ps", name="ps_g")
                for ko in range(4):
                    nc.tensor.matmul(
                        ps,
                        lhsT=wgt_sb[:, ko, mo * P : (mo + 1) * P],
                        rhs=xtq_sb[:, ko, :],
                        start=(ko == 0),
                        stop=(ko == 3),
                    )
                nc.vector.tensor_scalar_add(
                    out=z_sb[:, mo, :], in0=ps, scalar1=gb_sb[:, mo : mo + 1]
                )

            def attention_pair(
                hp, pre_pv=None, after_first_exp=None, after_first_mult=None
            ):
                heads = (2 * hp, 2 * hp + 1)
                pv_ps = {}
                for h in heads:
                    pv_ps[h] = ps_sm.tile(
                        [D + 1, NQ], F32, tag="ps", name=f"pv_{h}"
                    )
                for t in range(4):  # two nk-chunks of 128 per step
                    s_tiles = {}
                    for h in heads:
                        s_tiles[h] = ps_s.tile(
                            [P, 2, 512], F32, tag="s", name=f"s_{h}_{t}"
                        )
                    # j outer / h inner: the two heads' K=64 matmuls are
                    # adjacent in the PE stream -> row-packed concurrency
                    for j in range(2):
                        c = 2 * t + j
                        for h in heads:
                            d0 = (h % 2) * D
                            mo = h // 2
                            nc.tensor.matmul(
                                s_tiles[h][:, j, :],
                                lhsT=kt_sb[d0 : d0 + D, mo, c * P : (c + 1) * P],
                                rhs=qt_sb[d0 : d0 + D, mo, :],
                                start=True,
                                stop=True,
                            )
                    e_tiles = {}
                    for h in heads:
                        e = etmp.tile(
                            [P, 2, 512], BF16, tag="e", name=f"e_{h}_{t}", bufs=6
                        )
                        e_tiles[h] = e
                        nc.scalar.activation(out=e, in_=s_tiles[h], func=AF.Exp)
                    if t == 0 and after_first_exp is not None:
                        after_first_exp()
                    for h in heads:
                        nc.vector.tensor_tensor(
                            e_tiles[h],
                            e_tiles[h],
                            ebt_sb[:, h * 8 + 2 * t : h * 8 + 2 * t + 2, :],
                            mybir.AluOpType.mult,
                        )
                    if t == 0 and after_first_mult is not None:
                        after_first_mult()
                    if pre_pv is not None:
                        pre_pv(t)
                    for h in heads:
                        for j in range(2):
                            c = 2 * t + j
                            nc.tensor.matmul(
                                pv_ps[h],
                                lhsT=vaug_sb[
                                    :, c, h * (D + 1) : (h + 1) * (D + 1)
                                ],
                                rhs=e_tiles[h][:, j, :],
                                start=(c == 0),
                                stop=(c == 7),
                            )
                return pv_ps

            def oraw_evac(pv_ps):
                # one DVE copy per head grabs o rows AND the den row.
                # On DVE (not ACT): an ACT-resident copy head-of-line
                # blocks the next pair's exps behind PV completion.
                for h, ps in pv_ps.items():
                    nc.vector.tensor_copy(out=oraw_sb[:, h, :], in_=ps)

            def norm_gather(h0, cnt=4):
                den4 = singles.tile([cnt, NQ], BF16, name=f"den4_{h0}")
                nc.sync.dma_start(
                    out=den4, in_=oraw_sb[D : D + 1, h0 : h0 + cnt, :]
                )
                return den4

            def norm_apply(h0, den4, cnt=4):
                """Normalize heads h0..h0+cnt (one DVE reciprocal for all)."""
                recip4 = singles.tile([cnt, NQ], BF16, name=f"recip4_{h0}")
                with nc.allow_low_precision(
                    reason="softmax denom recip in bf16"
                ):
                    nc.vector.reciprocal(out=recip4, in_=den4)
                for i in range(cnt):
                    h = h0 + i
                    rbc_ps = ps_sm.tile([D, NQ], F32, tag="ps", name=f"rbc_{h}")
                    nc.tensor.matmul(
                        rbc_ps,
                        lhsT=sel8_sb[0:cnt, i * D : (i + 1) * D],
                        rhs=recip4,
                        start=True,
                        stop=True,
                    )
                    rbc_sb = vtmp.tile([D, NQ], BF16, tag="rbc", name=f"rbc_sb_{h}", bufs=4)
                    nc.scalar.copy(out=rbc_sb, in_=rbc_ps)
                    nc.vector.tensor_tensor(
                        osc_sb[:, h, :],
                        oraw_sb[0:D, h, :],
                        rbc_sb,
                        mybir.AluOpType.mult,
                    )
                # relocate: even heads -> partitions 0-63, odd -> 64-127
                mo0 = h0 // 2
                nmo = cnt // 2
                evens = osc_sb[:, h0 : h0 + cnt, :].rearrange(
                    "p (m t) q -> p m t q", t=2
                )
                nc.sync.dma_start(
                    out=ofin_sb[0:D, mo0 : mo0 + nmo, :], in_=evens[:, :, 0, :]
                )
                nc.sync.dma_start(
                    out=ofin_sb[D:P, mo0 : mo0 + nmo, :], in_=evens[:, :, 1, :]
                )

            tmpo_sb = singles.tile([P, 4, NQ], F32)  # to_out ko 0-1 partial

            def toout_p1(mo):
                """to_out over ko 0-1 (heads 0-3; ready after reloc of 0-3),
                staged to SBUF with bo folded in."""
                ps = ps_sm.tile([P, 512], F32, tag="ps", name="ps_o1")
                for ko in range(2):
                    nc.tensor.matmul(
                        ps,
                        lhsT=wot_sb[:, ko, mo * P : (mo + 1) * P],
                        rhs=ofin_sb[:, ko, :],
                        start=(ko == 0),
                        stop=(ko == 1),
                    )
                nc.vector.tensor_scalar_add(
                    out=tmpo_sb[:, mo, :], in0=ps, scalar1=bo_sb[:, mo : mo + 1]
                )

            def toout_p2(mo):
                ps = ps_sm.tile([P, 512], F32, tag="ps", name="ps_o2")
                for ko in range(2, 4):
                    nc.tensor.matmul(
                        ps,
                        lhsT=wot_sb[:, ko, mo * P : (mo + 1) * P],
                        rhs=ofin_sb[:, ko, :],
                        start=(ko == 2),
                        stop=(ko == 3),
                    )
                tmp = etmp.tile([P, NQ], F32, tag="otmp", name="otmp")
                nc.vector.tensor_add(out=tmp, in0=ps, in1=tmpo_sb[:, mo, :])
                nc.vector.tensor_tensor(
                    outf_sb[:, mo, :],
                    tmp,
                    gate_sb[:, mo, :],
                    mybir.AluOpType.mult,
                )
                nc.sync.dma_start(out=out_e[:, mo, :], in_=outf_sb[:, mo, :])

            # schedule (v6 phasing): pair 0 early; bulk projections dense;
            # gate + to_out in the tail
            def pair0_phaseA():
                """Logits + exps for heads 0/1, emitted right after kt0/qt0:
                ACT is busy from ~13us. e-tiles come from a DEDICATED pool
                (tag e0) so holding all 8 until phaseB never starves the
                later pairs' exp pipeline."""
                e_list = []
                for t in range(4):
                    s_tiles = {}
                    for h in (0, 1):
                        s_tiles[h] = ps_s.tile(
                            [P, 2, 512], F32, tag="s", name=f"s_{h}_{t}"
                        )
                    for j in range(2):
                        c = 2 * t + j
                        for h in (0, 1):
                            d0 = (h % 2) * D
                            nc.tensor.matmul(
                                s_tiles[h][:, j, :],
                                lhsT=kt_sb[d0 : d0 + D, 0, c * P : (c + 1) * P],
                                rhs=qt_sb[d0 : d0 + D, 0, :],
                                start=True,
                                stop=True,
                            )
                    for h in (0, 1):
                        e = etmp.tile(
                            [P, 2, 512], BF16, tag="e0", name=f"e0_{h}_{t}",
                            bufs=8,
                        )
                        nc.scalar.activation(out=e, in_=s_tiles[h], func=AF.Exp)
                        e_list.append((h, t, e))
                return e_list

            def pair0_phaseB(e_list):
                pv_ps = {}
                for h in (0, 1):
                    pv_ps[h] = ps_sm.tile(
                        [D + 1, NQ], F32, tag="ps", name=f"pv_{h}"
                    )
                by_ht = {(h, t): e for h, t, e in e_list}
                for t in range(4):
                    for h in (0, 1):
                        nc.vector.tensor_tensor(
                            by_ht[h, t],
                            by_ht[h, t],
                            ebt_sb[:, h * 8 + 2 * t : h * 8 + 2 * t + 2, :],
                            mybir.AluOpType.mult,
                        )
                    for h in (0, 1):
                        for j in range(2):
                            c = 2 * t + j
                            nc.tensor.matmul(
                                pv_ps[h],
                                lhsT=vaug_sb[
                                    :, c, h * (D + 1) : (h + 1) * (D + 1)
                                ],
                                rhs=by_ht[h, t][:, j, :],
                                start=(c == 0),
                                stop=(c == 7),
                            )
                return pv_ps

            kt_proj(0)
            qt_proj(0)
            e0 = pair0_phaseA()
            for c in range(8):
                v_proj(c)
            for mo in range(1, 4):
                kt_proj(mo)
                qt_proj(mo)
            pv0 = pair0_phaseB(e0)
            oraw_evac(pv0)
            pv1 = attention_pair(1)
            oraw_evac(pv1)
            denA = norm_gather(0, 4)
            pv2 = attention_pair(
                2, after_first_mult=lambda: norm_apply(0, denA, 4)
            )
            oraw_evac(pv2)
            pv3 = attention_pair(3)
            oraw_evac(pv3)
            # gate + to_out ko0-1 emitted BEFORE norm-B: their deps are
            # ready, so they fill PE/ACT during norm-B's gather + recip
            for mo in range(4):
                gate_proj(mo)
            nc.scalar.activation(out=gate_sb, in_=z_sb, func=AF.Tanh, scale=0.5)
            nc.vector.tensor_scalar(
                out=gate_sb,
                in0=gate_sb,
                scalar1=0.5,
                scalar2=0.5,
                op0=mybir.AluOpType.mult,
                op1=mybir.AluOpType.add,
            )
            for mo in range(4):
                toout_p1(mo)
            norm_apply(4, norm_gather(4, 4), 4)
            for mo in range(4):
                toout_p2(mo)

    nc.compile()
    return nc


def make_in_maps(q_x, attn_bias, Wq, bq, Wk, Wv, Wo, bo, Wg, bg, gating_bias):
    import ml_dtypes

    bf16 = ml_dtypes.bfloat16
    scale = np.float32(D) ** -0.5

    def swz(a2d):
        """[512, M] -> [128, 4, M] SBUF layout (partition-inner on dim 0)."""
        m = a2d.shape[1]
        return np.ascontiguousarray(a2d.reshape(4, P, m).transpose(1, 0, 2))

    wqt = swz(Wq.T.astype(np.float32) * scale).astype(bf16)
    wkt = swz(np.asarray(Wk.T, dtype=np.float32)).astype(bf16)
    wvt = swz(np.asarray(Wv.T, dtype=np.float32)).astype(bf16)
    wot = swz(np.asarray(Wo.T, dtype=np.float32)).astype(bf16)
    wgt = swz(np.asarray(Wg.T, dtype=np.float32)).astype(bf16)
    bqs = np.ascontiguousarray((bq * scale).reshape(4, P).T).astype(np.float32)
    bo_ = np.ascontiguousarray(np.asarray(bo).reshape(4, P).T).astype(np.float32)
    gb = np.ascontiguousarray((bg + gating_bias).reshape(4, P).T).astype(np.float32)
    sel8 = np.repeat(np.eye(H, dtype=np.float32), D, axis=1).astype(bf16)

    in_maps = []
    for c in range(8):
        b, half = c // 2, c % 2
        rows = slice(half * NQ, (half + 1) * NQ)
        x = np.asarray(q_x[b], dtype=np.float32)  # [N, CQ]
        xt = swz(x.T).astype(bf16)  # [128, 4, N]
        xtq = swz(np.ascontiguousarray(x[rows].T)).astype(bf16)
        # ebt[p, h*8+c, q] = exp(bias[b, h, rows, :]).T[c*128+p, q]
        eb = np.exp(np.asarray(attn_bias[b, :, rows, :], dtype=np.float32))
        ebt = np.ascontiguousarray(
            eb.transpose(0, 2, 1).reshape(H, 8, P, NQ).transpose(2, 0, 1, 3)
        ).reshape(P, H * 8, NQ).astype(bf16)
        in_maps.append(
            {
                "xt": xt,
                "xtq": xtq,
                "ebt": ebt,
                "wqt": wqt,
                "wkt": wkt,
                "wvt": wvt,
                "wot": wot,
                "wgt": wgt,
                "bqs": bqs,
                "bo": bo_,
                "gb": gb,
                "sel8": sel8,
            }
        )
    return in_maps


_NC_CACHE = None


def kernel(**inputs) -> np.ndarray:
    global _NC_CACHE
    from concourse.bass_utils import run_bass_kernel_spmd

    if _NC_CACHE is None:
        _NC_CACHE = build_nc()
    nc = _NC_CACHE
    in_maps = make_in_maps(**inputs)
    trace = bool(int(os.environ.get("BASS_KERNEL_TRACE", "0")))
    last_exc = None
    for attempt in range(3):
        try:
            res = run_bass_kernel_spmd(nc, in_maps, list(range(8)), trace=trace)
            break
        except Exception as exc:  # transient NRT/axon device hiccups
            last_exc = exc
            import time

            time.sleep(10 * (attempt + 1))
    else:
        raise last_exc
    kernel.last_result = res
    out = np.empty((B, N, CQ), dtype=np.float32)
    for c in range(8):
        b, half = c // 2, c % 2
        # res "out" is [128, 4, NQ]: out^T[cq=o*128+i, q] at [i, o, q]
        o = res.results[c]["out"]
        out[b, half * NQ : (half + 1) * NQ, :] = (
            o.transpose(1, 0, 2).reshape(CQ, NQ).T
        )
    return out



# revision 3
# speedup vs baseline: 1.1797x; 1.1797x over previous
"""Trainium2 Bass kernel for nn_Attention_79164837199973 (v7).

Bias-augmented multi-head self-attention with sigmoid gating.
B=4, N=1024, CQ=CH=512, H=8, D=64.

Sharding (8 cores, no collectives): core c -> batch b=c//2, query-row half
r=c%2 (512 rows). Keys are PERMUTED per core (own query-row half first) so
the q-projection reads a prefix slice of the same x^T tile -- no separate
xtq DMA. kt/ebt/vaug all use the permuted key order (softmax sum order is
irrelevant).

v7 changes vs v6 (trace-driven):
  - HAM warmth: the v6 schedule had >3.4us PE-idle windows; the PE spent
    ~38us re-throttled at 1.2 GHz (MM 630ns vs 379 warm). v7 interleaves
    projection/PV/gate/to_out matmuls as fillers inside the exp-paced
    attention pairs so the PE never idles long enough to re-throttle.
  - engine balance: ACT = exps + qt0 evac + oraw evacs + tanh-gate
    (tanh shares the exp table set; sigmoid does NOT -- gate =
    0.5*tanh(0.5 z + 0.5 gb)+0.5). DVE = kt/v/qt1-3 evacs + bias-mults +
    norm chain + to_out fusion.
  - norm: reciprocal_approx_fast on f32 (0.67us vs 3.3us DVE reciprocal),
    rbc broadcast matmul result multiplied straight from PSUM (kills the
    ACT copy). Norm in 3 batches (h0-3 mid, h4-5 late-mid, h6-7 tail).
  - to_out: all 4 ko accumulate in ONE PSUM bank per mo at the tail, then
    a single scalar_tensor_tensor (ps + bo) * gate evacuation.
  - ebt DMA in 16 chunks (pair,t)-ordered for arrival pacing.
"""

import os
import sys

sys.path.insert(0, "/opt/trn_rl_repo")

import numpy as np

import concourse.bass as bass
import concourse.tile as tile
from concourse import bacc, mybir

B, N, CQ, CH, H = 4, 1024, 512, 512, 8
D = CH // H  # 64
NQ = N // 2  # 512 query rows per core
P = 128
F32 = mybir.dt.float32
BF16 = mybir.dt.bfloat16
AF = mybir.ActivationFunctionType
OP = mybir.AluOpType


def build_nc():
    nc = bacc.Bacc("TRN2", target_bir_lowering=False, debug=False, num_devices=8)

    # ---- DRAM parameters, already in SBUF layout (host pre-swizzled) ----
    xt_e = nc.declare_dram_parameter("xt", [P, 4, N], BF16, isOutput=False)
    ebt_e = nc.declare_dram_parameter("ebt", [P, H * 8, NQ], BF16, isOutput=False)
    wqt_e = nc.declare_dram_parameter("wqt", [P, 4, CH], BF16, isOutput=False)
    wkt_e = nc.declare_dram_parameter("wkt", [P, 4, CH], BF16, isOutput=False)
    wvt_e = nc.declare_dram_parameter("wvt", [P, 4, CH], BF16, isOutput=False)
    wot_e = nc.declare_dram_parameter("wot", [P, 4, CQ], BF16, isOutput=False)
    wgt_e = nc.declare_dram_parameter("wgt", [P, 4, CQ], BF16, isOutput=False)
    bqs_e = nc.declare_dram_parameter("bqs", [P, 4], F32, isOutput=False)
    bo_e = nc.declare_dram_parameter("bo", [P, 4], F32, isOutput=False)
    gb_e = nc.declare_dram_parameter("gb", [P, 4], F32, isOutput=False)
    sel8_e = nc.declare_dram_parameter("sel8", [H, H * D], F32, isOutput=False)
    out_e = nc.declare_dram_parameter("out", [P, 4, NQ], F32, isOutput=True)

    with tile.TileContext(nc) as tc:
        with (
            tc.tile_pool(name="singles", bufs=1) as singles,
            tc.tile_pool(name="etmp", bufs=3) as etmp,
            tc.tile_pool(name="ps_s", bufs=2, space="PSUM") as ps_s,
            tc.tile_pool(name="ps_pv", bufs=2, space="PSUM") as ps_pv,
            tc.tile_pool(name="ps_pr", bufs=2, space="PSUM") as ps_pr,
        ):
            # ---- persistent SBUF tiles ----
            xt_sb = singles.tile([P, 4, N], BF16)
            wqt_sb = singles.tile([P, 4, CH], BF16)
            wkt_sb = singles.tile([P, 4, CH], BF16)
            wvt_sb = singles.tile([P, 4, CH], BF16)
            wot_sb = singles.tile([P, 4, CQ], BF16)
            wgt_sb = singles.tile([P, 4, CQ], BF16)
            bqs_sb = singles.tile([P, 4], F32)
            bo_sb = singles.tile([P, 4], F32)
            gb_sb = singles.tile([P, 4], F32)
            sel8_sb = singles.tile([H, H * D], F32)
            ebt_sb = singles.tile([P, H * 8, NQ], BF16)
            kt_sb = singles.tile([P, 4, N], BF16)
            qt_sb = singles.tile([P, 4, NQ], BF16)
            vaug_sb = singles.tile([P, 8, H * (D + 1)], BF16)
            oraw_sb = singles.tile([D + 1, H, NQ], BF16)  # 0-63 o, row 64 den
            osc_sb = singles.tile([D, H, NQ], BF16)  # normalized o
            ofin_sb = singles.tile([P, 4, NQ], BF16)  # head-merged o^T
            gate_sb = singles.tile([P, 4, NQ], BF16)
            outf_sb = singles.tile([P, 4, NQ], F32)
            warm_sb = singles.tile([1, 8], F32)
            warmo_sb = singles.tile([1, 8], BF16)

            # force the exp table load off the critical path (first ACT op)
            nc.vector.memset(warm_sb, 0.0)
            nc.scalar.activation(out=warmo_sb, in_=warm_sb, func=AF.Exp)

            # ---- input DMAs: FIFO order = priority order ----
            nc.sync.dma_start(out=wkt_sb, in_=wkt_e[:, :, :])
            nc.sync.dma_start(out=xt_sb[:, :, 0:NQ], in_=xt_e[:, :, 0:NQ])
            nc.sync.dma_start(out=wqt_sb, in_=wqt_e[:, :, :])
            nc.sync.dma_start(out=bqs_sb, in_=bqs_e[:, :])
            nc.sync.dma_start(out=xt_sb[:, :, NQ:N], in_=xt_e[:, :, NQ:N])
            nc.sync.dma_start(out=wvt_sb, in_=wvt_e[:, :, :])
            nc.sync.dma_start(out=gb_sb, in_=gb_e[:, :])
            nc.sync.dma_start(out=sel8_sb, in_=sel8_e[:, :])
            nc.sync.dma_start(out=bo_sb, in_=bo_e[:, :])

            def ebt_load(p, t):
                r0 = p * 16 + t * 4
                nc.sync.dma_start(
                    out=ebt_sb[:, r0 : r0 + 4, :], in_=ebt_e[:, r0 : r0 + 4, :]
                )

            for p in range(3):
                for t in range(4):
                    ebt_load(p, t)
            nc.sync.dma_start(out=wgt_sb, in_=wgt_e[:, :, :])
            for t in range(4):
                ebt_load(3, t)
            nc.sync.dma_start(out=wot_sb, in_=wot_e[:, :, :])

            # ones column of v_aug (col D of each head's 65-wide group)
            nc.vector.memset(
                vaug_sb.rearrange("p c (h e) -> p c h e", h=H)[:, :, :, D : D + 1],
                1.0,
            )

            # ---------------- projection helpers ----------------
            def kt_proj(mo, no):
                ps = ps_pr.tile([P, 512], F32, tag="pr", name=f"ps_k{mo}{no}")
                for ko in range(4):
                    nc.tensor.matmul(
                        ps,
                        lhsT=wkt_sb[:, ko, mo * P : (mo + 1) * P],
                        rhs=xt_sb[:, ko, no * 512 : (no + 1) * 512],
                        start=(ko == 0),
                        stop=(ko == 3),
                    )
                nc.vector.tensor_copy(
                    out=kt_sb[:, mo, no * 512 : (no + 1) * 512], in_=ps
                )

            def qt_proj(mo, act_evac=False):
                ps = ps_pr.tile([P, 512], F32, tag="pr", name=f"ps_q{mo}")
                for ko in range(4):
                    nc.tensor.matmul(
                        ps,
                        lhsT=wqt_sb[:, ko, mo * P : (mo + 1) * P],
                        rhs=xt_sb[:, ko, 0:NQ],
                        start=(ko == 0),
                        stop=(ko == 3),
                    )
                if act_evac:
                    nc.scalar.activation(
                        out=qt_sb[:, mo, :],
                        in_=ps,
                        func=AF.Identity,
                        bias=bqs_sb[:, mo : mo + 1],
                    )
                else:
                    nc.vector.tensor_scalar_add(
                        out=qt_sb[:, mo, :], in0=ps, scalar1=bqs_sb[:, mo : mo + 1]
                    )

            def v_proj(c):
                ps = ps_pr.tile([P, 512], F32, tag="pr", name=f"ps_v{c}")
                for ko in range(4):
                    nc.tensor.matmul(
                        ps,
                        lhsT=xt_sb[:, ko, c * P : (c + 1) * P],
                        rhs=wvt_sb[:, ko, :],
                        start=(ko == 0),
                        stop=(ko == 3),
                    )
                dst = vaug_sb.rearrange("p c (h e) -> p c h e", h=H)[:, c, :, 0:D]
                nc.vector.tensor_copy(
                    out=dst, in_=ps.rearrange("p (h d) -> p h d", h=H)
                )

            # ---------------- attention building blocks ----------------
            def logits_group(p, t, tag, bufs):
                """4 logit MMs (j-outer, h-inner for row-group packing) +
                2 exps. Returns {h: e_tile}."""
                heads = (2 * p, 2 * p + 1)
                s_tiles = {}
                for h in heads:
                    s_tiles[h] = ps_s.tile(
                        [P, 2, 512], F32, tag="s", name=f"s_{h}_{t}"
                    )
                for j in range(2):
                    c = 2 * t + j
                    for h in heads:
                        d0 = (h % 2) * D
                        nc.tensor.matmul(
                            s_tiles[h][:, j, :],
                            lhsT=kt_sb[d0 : d0 + D, p, c * P : (c + 1) * P],
                            rhs=qt_sb[d0 : d0 + D, p, :],
                            start=True,
                            stop=True,
                        )
                e_tiles = {}
                for h in heads:
                    e = etmp.tile(
                        [P, 2, 512], BF16, tag=tag, name=f"e_{h}_{t}", bufs=bufs
                    )
                    e_tiles[h] = e
                    nc.scalar.activation(out=e, in_=s_tiles[h], func=AF.Exp)
                return e_tiles

            def mults(p, t, e_tiles):
                for h in (2 * p, 2 * p + 1):
                    r0 = p * 16 + t * 4 + (h % 2) * 2
                    nc.vector.tensor_tensor(
                        e_tiles[h],
                        e_tiles[h],
                        ebt_sb[:, r0 : r0 + 2, :],
                        OP.mult,
                    )

            def pv_group(p, t, e_tiles, pv_ps):
                for h in (2 * p, 2 * p + 1):
                    for j in range(2):
                        c = 2 * t + j
                        nc.tensor.matmul(
                            pv_ps[h],
                            lhsT=vaug_sb[:, c, h * (D + 1) : (h + 1) * (D + 1)],
                            rhs=e_tiles[h][:, j, :],
                            start=(c == 0),
                            stop=(c == 7),
                        )

            def pv_alloc(p):
                return {
                    h: ps_pv.tile([D + 1, NQ], F32, tag="pv", name=f"pv_{h}")
                    for h in (2 * p, 2 * p + 1)
                }

            def oraw_evac(pv_ps):
                # ACT copies, emitted at pair boundaries (deps already met)
                for h, ps in pv_ps.items():
                    nc.scalar.copy(out=oraw_sb[:, h, :], in_=ps)

            # ---------------- softmax normalization ----------------
            def norm_gather(h0, cnt):
                den_bf = singles.tile([cnt, NQ], BF16, name=f"denb_{h0}")
                nc.sync.dma_start(
                    out=den_bf, in_=oraw_sb[D : D + 1, h0 : h0 + cnt, :]
                )
                return den_bf

            def norm_recip(h0, cnt, den_bf):
                den_f = singles.tile([cnt, NQ], F32, name=f"denf_{h0}")
                recipf = singles.tile([cnt, NQ], F32, name=f"recipf_{h0}")
                nc.vector.tensor_copy(out=den_f, in_=den_bf)
                nc.vector.reciprocal_approx_fast(out=recipf, in_=den_f)
                return recipf

            def norm_head(h0, cnt, i, recipf):
                """Broadcast 1/den for head h0+i over 64 partitions (PE) and
                scale oraw -> osc (DVE, straight from PSUM)."""
                h = h0 + i
                rbc = ps_pr.tile([D, NQ], F32, tag="pr", name=f"rbc_{h}")
                nc.tensor.matmul(
                    rbc,
                    lhsT=sel8_sb[0:cnt, i * D : (i + 1) * D],
                    rhs=recipf,
                    start=True,
                    stop=True,
                )
                nc.vector.tensor_tensor(
                    osc_sb[:, h, :], oraw_sb[0:D, h, :], rbc, OP.mult
                )

            def norm_reloc(h0, cnt):
                # even heads -> partitions 0-63, odd -> 64-127
                mo0 = h0 // 2
                nmo = cnt // 2
                evens = osc_sb[:, h0 : h0 + cnt, :].rearrange(
                    "p (m t) q -> p m t q", t=2
                )
                nc.sync.dma_start(
                    out=ofin_sb[0:D, mo0 : mo0 + nmo, :], in_=evens[:, :, 0, :]
                )
                nc.sync.dma_start(
                    out=ofin_sb[D:P, mo0 : mo0 + nmo, :], in_=evens[:, :, 1, :]
                )

            # ---------------- gate + to_out ----------------
            def gate_proj(mo):
                ps = ps_pr.tile([P, 512], F32, tag="pr", name=f"ps_g{mo}")
                for ko in range(4):
                    nc.tensor.matmul(
                        ps,
                        lhsT=wgt_sb[:, ko, mo * P : (mo + 1) * P],
                        rhs=xt_sb[:, ko, 0:NQ],
                        start=(ko == 0),
                        stop=(ko == 3),
                    )
                # gate = 0.5*tanh(0.5*z + 0.5*gb) + 0.5 == sigmoid(z + gb)
                # (tanh shares the exp table set; sigmoid would thrash it)
                nc.scalar.activation(
                    out=gate_sb[:, mo, :],
                    in_=ps,
                    func=AF.Tanh,
                    bias=gb_sb[:, mo : mo + 1],
                    scale=0.5,
                )
                nc.vector.tensor_scalar(
                    out=gate_sb[:, mo, :],
                    in0=gate_sb[:, mo, :],
                    scalar1=0.5,
                    scalar2=0.5,
                    op0=OP.mult,
                    op1=OP.add,
                )

            toout_ps = {}

            def toout_partial(mo, kos):
                if mo not in toout_ps:
                    toout_ps[mo] = ps_s.tile(
                        [P, 512], F32, tag="s", name=f"to_{mo}"
                    )
                for ko in kos:
                    nc.tensor.matmul(
                        toout_ps[mo],
                        lhsT=wot_sb[:, ko, mo * P : (mo + 1) * P],
                        rhs=ofin_sb[:, ko, :],
                        start=(ko == 0),
                        stop=(ko == 3),
                    )

            def toout_finish(mo):
                # out = (ps + bo) * gate, single fused DVE op, then DMA out
                nc.vector.scalar_tensor_tensor(
                    out=outf_sb[:, mo, :],
                    in0=toout_ps[mo],
                    scalar=bo_sb[:, mo : mo + 1],
                    in1=gate_sb[:, mo, :],
                    op0=OP.add,
                    op1=OP.mult,
                )
                nc.sync.dma_start(out=out_e[:, mo, :], in_=outf_sb[:, mo, :])

            # ================= schedule =================
            kt_proj(0, 0)
            qt_proj(0, act_evac=True)
            kt_proj(0, 1)

            # pair 0 logits+exps early (ACT busy ASAP); PV deferred past v
            e0 = {}
            for t in range(4):
                e0[t] = logits_group(0, t, tag="e0", bufs=8)
            for c in range(4):
                v_proj(c)
            kt_proj(1, 0)
            kt_proj(1, 1)
            for c in range(4, 8):
                v_proj(c)
            qt_proj(1)

            pv0 = pv_alloc(0)
            for t in range(4):
                mults(0, t, e0[t])
                pv_group(0, t, e0[t], pv0)

            def run_pair(p, fillers, after_first_exp=None, after_first_mult=None):
                pv = pv_alloc(p)
                for t in range(4):
                    e = logits_group(p, t, tag="e", bufs=6)
                    if t == 0 and after_first_exp is not None:
                        after_first_exp()
                    mults(p, t, e)
                    if t == 0 and after_first_mult is not None:
                        after_first_mult()
                    if t in fillers:
                        fillers[t]()
                    pv_group(p, t, e, pv)
                return pv

            pv1 = run_pair(
                1,
                fillers={
                    0: lambda: kt_proj(2, 0),
                    1: lambda: kt_proj(2, 1),
                    2: lambda: qt_proj(2),
                },
                after_first_exp=lambda: oraw_evac(pv0),
            )

            state = {}

            def pair2_mid():
                oraw_evac(pv1)
                state["denA"] = norm_gather(0, 4)

            pv2 = run_pair(
                2,
                fillers={
                    0: lambda: kt_proj(3, 0),
                    1: lambda: kt_proj(3, 1),
                    2: lambda: qt_proj(3),
                    3: lambda: state.update(
                        recipA=norm_recip(0, 4, state["denA"])
                    ),
                },
                after_first_exp=pair2_mid,
            )

            def pair3_mid():
                oraw_evac(pv2)
                state["denB1"] = norm_gather(4, 2)

            def normA(i):
                return lambda: norm_head(0, 4, i, state["recipA"])

            pv3 = run_pair(
                3,
                fillers={
                    0: normA(0),
                    1: normA(1),
                    2: lambda: (
                        norm_head(0, 4, 2, state["recipA"]),
                        norm_head(0, 4, 3, state["recipA"]),
                        norm_reloc(0, 4),
                        state.update(recipB1=norm_recip(4, 2, state["denB1"])),
                    ),
                    3: lambda: (
                        norm_head(4, 2, 0, state["recipB1"]),
                        norm_head(4, 2, 1, state["recipB1"]),
                        norm_reloc(4, 2),
                    ),
                },
                after_first_exp=pair3_mid,
            )

            # ---- tail ----
            # only 2 to_out PSUM tiles can be live (ps_s has 2 slots): mo0/mo1
            # partials start early (ko0-2 need only norm A+B1); mo2/mo3 run
            # full ko0-3 once slots free after the mo0/mo1 finishes.
            oraw_evac(pv3)
            denB2 = norm_gather(6, 2)
            gate_proj(0)
            toout_partial(0, (0, 1, 2))
            gate_proj(1)
            toout_partial(1, (0, 1, 2))
            recipB2 = norm_recip(6, 2, denB2)
            gate_proj(2)
            norm_head(6, 2, 0, recipB2)
            norm_head(6, 2, 1, recipB2)
            norm_reloc(6, 2)
            gate_proj(3)
            toout_partial(0, (3,))
            toout_finish(0)
            toout_partial(1, (3,))
            toout_finish(1)
            toout_partial(2, (0, 1, 2, 3))
            toout_finish(2)
            toout_partial(3, (0, 1, 2, 3))
            toout_finish(3)

    nc.compile()
    return nc


def make_in_maps(q_x, attn_bias, Wq, bq, Wk, Wv, Wo, bo, Wg, bg, gating_bias):
    import ml_dtypes

    bf16 = ml_dtypes.bfloat16
    scale = np.float32(D) ** -0.5

    def swz(a2d):
        """[512, M] -> [128, 4, M] SBUF layout (partition-inner on dim 0)."""
        m = a2d.shape[1]
        return np.ascontiguousarray(a2d.reshape(4, P, m).transpose(1, 0, 2))

    wqt = swz(Wq.T.astype(np.float32) * scale).astype(bf16)
    wkt = swz(np.asarray(Wk.T, dtype=np.float32)).astype(bf16)
    wvt = swz(np.asarray(Wv.T, dtype=np.float32)).astype(bf16)
    wot = swz(np.asarray(Wo.T, dtype=np.float32)).astype(bf16)
    wgt = swz(np.asarray(Wg.T, dtype=np.float32)).astype(bf16)
    bqs = np.ascontiguousarray((bq * scale).reshape(4, P).T).astype(np.float32)
    bo_ = np.ascontiguousarray(np.asarray(bo).reshape(4, P).T).astype(np.float32)
    gb = np.ascontiguousarray(
        (0.5 * (bg + gating_bias)).reshape(4, P).T
    ).astype(np.float32)
    sel8 = np.repeat(np.eye(H, dtype=np.float32), D, axis=1)

    in_maps = []
    for c in range(8):
        b, half = c // 2, c % 2
        o0, o1 = half * NQ, (1 - half) * NQ
        x = np.asarray(q_x[b], dtype=np.float32)  # [N, CQ]
        # keys permuted: own query-row half first (q reads prefix of xt)
        xp = np.concatenate([x[o0 : o0 + NQ], x[o1 : o1 + NQ]], axis=0)
        xt = swz(np.ascontiguousarray(xp.T)).astype(bf16)  # [128, 4, N]
        # ebt[p, pair*16 + t*4 + h01*2 + j, q] = exp(bias)[2*pair+h01,
        #   perm_key[(2t+j)*128+p], own_row q]
        eb = np.exp(np.asarray(attn_bias[b, :, o0 : o0 + NQ, :], np.float32))
        ebp = np.concatenate(
            [eb[:, :, o0 : o0 + NQ], eb[:, :, o1 : o1 + NQ]], axis=2
        )  # [H, q, k(perm)]
        ebt = (
            ebp.transpose(0, 2, 1)  # [H, k, q]
            .reshape(4, 2, 4, 2, P, NQ)  # [pair, h01, t, j, p, q]
            .transpose(4, 0, 2, 1, 3, 5)  # [p, pair, t, h01, j, q]
            .reshape(P, H * 8, NQ)
        )
        ebt = np.ascontiguousarray(ebt).astype(bf16)
        in_maps.append(
            {
                "xt": xt,
                "ebt": ebt,
                "wqt": wqt,
                "wkt": wkt,
                "wvt": wvt,
                "wot": wot,
                "wgt": wgt,
                "bqs": bqs,
                "bo": bo_,
                "gb": gb,
                "sel8": sel8,
            }
        )
    return in_maps


_NC_CACHE = None


def kernel(**inputs) -> np.ndarray:
    global _NC_CACHE
    from concourse.bass_utils import run_bass_kernel_spmd

    if _NC_CACHE is None:
        _NC_CACHE = build_nc()
    nc = _NC_CACHE
    in_maps = make_in_maps(**inputs)
    trace = bool(int(os.environ.get("BASS_KERNEL_TRACE", "0")))
    last_exc = None
    for attempt in range(3):
        try:
            res = run_bass_kernel_spmd(nc, in_maps, list(range(8)), trace=trace)
            break
        except Exception as exc:  # transient NRT/axon device hiccups
            last_exc = exc
            import time

            time.sleep(10 * (attempt + 1))
    else:
        raise last_exc
    kernel.last_result = res
    out = np.empty((B, N, CQ), dtype=np.float32)
    for c in range(8):
        b, half = c // 2, c % 2
        # res "out" is [128, 4, NQ]: out^T[cq=o*128+i, q] at [i, o, q]
        o = res.results[c]["out"]
        out[b, half * NQ : (half + 1) * NQ, :] = (
            o.transpose(1, 0, 2).reshape(CQ, NQ).T
        )
    return out
